# revision 1
# baseline (speedup 1.0000x reference)
"""ComplexAttention (B=2, T=2048, D=1024, H=16, Dh=64) on 8 TRN2 NeuronCores.

Sharding: core c -> batch b = c // 4, heads [4*(c%4), 4*(c%4)+4).
Each core computes its 4 heads' QKV projections (column-sharded), causal
complex attention, and a partial output projection (row-sharded). The host
sums the 4 partials per batch and adds the output bias.

Math notes:
  score = (qr kr^T + qi ki^T) / 8  ==  Qc Kc^T / 8  with Qc = [qr; qi] (128-d)
  -> contraction dim is exactly 128 = full PE partition dim.
  Attention is computed in the transposed domain: S^T[ktok, qtok] tiles,
  exp on ACT (no max subtraction needed: |S| <~ 3), causal mask via
  affine_select, unnormalized O^T = V^T-ish accumulation on PE, row sums
  l via ones-matmul, normalization by 1/l broadcast with a K=1 matmul.
"""

import math
from contextlib import ExitStack

import numpy as np

import concourse.bass as bass
import concourse.tile as tile
from concourse import bacc, mybir
from concourse.bass_utils import run_bass_kernel_spmd
from concourse.masks import make_identity

F32 = mybir.dt.float32
F32R = mybir.dt.float32r

# Full-problem config (hardcoded per harness contract).
CFG = dict(T=2048, D=1024, HPC=4, DH=64, TCH=256, QCH=512)
N_CORES = 8
B = 2
H_TOTAL = 16

# Flipped by test.py for profiling; harness path keeps these defaults.
TRACE = False
LAST = {}


# fp32r would stream at ~1 cyc/row vs 4 for fp32 (cost model: 486us vs
# 1331us/core), but walrus rejects this kernel's M=64 / tile_position /
# K=1 matmuls at fp32r (NCC_IXCG864), so ship exact fp32.
USE_F32R = False
MD = F32R if USE_F32R else F32


def _mm(x):
    return x


def _dm(ap):
    return ap.bitcast(F32R) if USE_F32R else ap


def build_program(cfg, num_devices=N_CORES, enable_asserts=False):
    """Build the per-core SPMD Bass program. Returns (nc, names) where names
    lists the input tensor names."""
    T, D, HPC, DH = cfg["T"], cfg["D"], cfg["HPC"], cfg["DH"]
    TCH, QCH = cfg["TCH"], cfg["QCH"]
    P = 128
    DT = D // P            # din tiles
    NCH = T // TCH         # phase-1 token chunks
    TS = TCH // P          # token subtiles per chunk
    KT = T // P            # key tiles
    QC = T // QCH          # phase-2 query chunks
    QKB = QCH // P         # key tiles per query chunk step
    CW = HPC * DH          # per-core qkv width
    NPAIR = HPC // 2
    scale = 1.0 / math.sqrt(DH)

    assert DH == 64 and P == 128 and CW % 128 == 0

    nc = bacc.Bacc(
        "TRN2",
        target_bir_lowering=False,
        debug=False,
        enable_asserts=enable_asserts,
        num_devices=num_devices,
    )

    # ---- DRAM I/O ----
    x_r = nc.dram_tensor("x_r", [T, D], F32, kind="ExternalInput").ap()
    x_i = nc.dram_tensor("x_i", [T, D], F32, kind="ExternalInput").ap()
    wq_r = nc.dram_tensor("wq_r", [D, CW], F32, kind="ExternalInput").ap()
    wq_i = nc.dram_tensor("wq_i", [D, CW], F32, kind="ExternalInput").ap()
    wk_r = nc.dram_tensor("wk_r", [D, CW], F32, kind="ExternalInput").ap()
    wk_i = nc.dram_tensor("wk_i", [D, CW], F32, kind="ExternalInput").ap()
    wv_r = nc.dram_tensor("wv_r", [D, CW], F32, kind="ExternalInput").ap()
    wv_i = nc.dram_tensor("wv_i", [D, CW], F32, kind="ExternalInput").ap()
    wo_r = nc.dram_tensor("wo_r", [CW, D], F32, kind="ExternalInput").ap()
    wo_i = nc.dram_tensor("wo_i", [CW, D], F32, kind="ExternalInput").ap()
    bq = nc.dram_tensor("bq", [P, HPC], F32, kind="ExternalInput").ap()
    bk = nc.dram_tensor("bk", [P, HPC], F32, kind="ExternalInput").ap()
    bv_r = nc.dram_tensor("bv_r", [1, CW], F32, kind="ExternalInput").ap()
    bv_i = nc.dram_tensor("bv_i", [1, CW], F32, kind="ExternalInput").ap()
    out_r = nc.dram_tensor("out_r", [T, D], F32, kind="ExternalOutput").ap()
    out_i = nc.dram_tensor("out_i", [T, D], F32, kind="ExternalOutput").ap()

    x_r_t = x_r.rearrange("(n p) d -> p n d", p=P)
    x_i_t = x_i.rearrange("(n p) d -> p n d", p=P)
    out_r_t = out_r.rearrange("(n p) d -> p n d", p=P)
    out_i_t = out_i.rearrange("(n p) d -> p n d", p=P)

    with tile.TileContext(nc) as tc, ExitStack() as octx:
        # ---- long-lived pools ----
        const = octx.enter_context(tc.tile_pool(name="const", bufs=1))
        opool = octx.enter_context(tc.tile_pool(name="opool", bufs=1))
        dram = octx.enter_context(tc.tile_pool(name="dram", bufs=1, space="DRAM"))

        ident = const.tile([P, P], F32)
        make_identity(nc, ident)
        # memset can't write f32r; stage f32 ones and cast via ACT copy
        ones_st = const.tile([P, P], F32)
        nc.vector.memset(ones_st, 1.0)
        ones_col = const.tile([P, 1], MD)   # lhsT for l = ones^T @ expS
        nc.scalar.activation(ones_col, ones_st[:, 0:1],
                             mybir.ActivationFunctionType.Copy)
        ones_row = const.tile([1, P], MD)   # lhsT for 1/l broadcast
        nc.scalar.activation(ones_row, ones_st[0:1, :],
                             mybir.ActivationFunctionType.Copy)
        bq_sb = const.tile([P, HPC], F32)
        nc.sync.dma_start(bq_sb, bq)
        bk_sb = const.tile([P, HPC], F32)
        nc.sync.dma_start(bk_sb, bk)
        bvr_sb = const.tile([1, CW], MD)
        nc.sync.dma_start(bvr_sb, _dm(bv_r))
        bvi_sb = const.tile([1, CW], MD)
        nc.sync.dma_start(bvi_sb, _dm(bv_i))

        # V stays SBUF-resident: [p, ktile, head*128 + (vr64|vi64)]
        v_sb = opool.tile([P, KT, HPC * P], MD)
        # O^T head-pair blocks, SBUF-resident into phase 3.
        # ORT[pair] rows: [vr_h_even(64) ; vr_h_odd(64)]
        # OIT[pair] rows: [vi_h_odd(64) ; vi_h_even(64)]  (host permutes wo_i)
        ort = [opool.tile([P, T], MD, name=f"ort{p}") for p in range(NPAIR)]
        oit = [opool.tile([P, T], MD, name=f"oit{p}") for p in range(NPAIR)]

        # DRAM scratch for Qc/Kc (d-major per head: [qr(64);qi(64)] x T)
        qt_d = dram.tile([HPC, P, T], MD)
        kt_d = dram.tile([HPC, P, T], MD)

        # ================= Phase 1: projections =================
        with ExitStack() as ctx:
            wpool = ctx.enter_context(tc.tile_pool(name="wpool", bufs=1))
            xin = ctx.enter_context(tc.tile_pool(name="xin", bufs=2))
            xtp = ctx.enter_context(tc.tile_pool(name="xtp", bufs=2))
            sqk = ctx.enter_context(tc.tile_pool(name="sqk", bufs=3))
            ps_t = ctx.enter_context(tc.tile_pool(name="ps_t", bufs=2, space="PSUM"))
            ps_qk = ctx.enter_context(tc.tile_pool(name="ps_qk", bufs=2, space="PSUM"))
            ps_v = ctx.enter_context(tc.tile_pool(name="ps_v", bufs=2, space="PSUM"))

            def load_w(ap_dram, name):
                w = wpool.tile([P, DT, CW], MD, name=name)
                nc.sync.dma_start(
                    w, _dm(ap_dram.rearrange("(t p) m -> p t m", p=P)))
                return w

            wq_r_sb = load_w(wq_r, "wq_r_sb")
            wq_i_sb = load_w(wq_i, "wq_i_sb")
            wk_r_sb = load_w(wk_r, "wk_r_sb")
            wk_i_sb = load_w(wk_i, "wk_i_sb")
            wv_r_sb = load_w(wv_r, "wv_r_sb")
            wv_i_sb = load_w(wv_i, "wv_i_sb")

            for tch in range(NCH):
                xr_c = xin.tile([P, TS, D], F32, name="xr_c")
                nc.sync.dma_start(xr_c, x_r_t[:, tch * TS:(tch + 1) * TS, :])
                xi_c = xin.tile([P, TS, D], F32, name="xi_c")
                nc.sync.dma_start(xi_c, x_i_t[:, tch * TS:(tch + 1) * TS, :])

                # transpose x chunk -> x^T [din, tok]
                xrT = xtp.tile([P, DT, TCH], MD, name="xrT")
                xiT = xtp.tile([P, DT, TCH], MD, name="xiT")
                for src, dst in ((xr_c, xrT), (xi_c, xiT)):
                    for s in range(TS):
                        for d in range(DT):
                            pt = ps_t.tile([P, P], F32, name="pt")
                            nc.tensor.transpose(
                                pt, src[:, s, d * P:(d + 1) * P], ident)
                            nc.any.tensor_copy(
                                out=dst[:, d, s * P:(s + 1) * P], in_=pt)

                # Q/K d-major per head: psum [qr_h(64); qi_h(64)] x TCH
                for h in range(HPC):
                    for (wr, wi, bias, dstd) in (
                        (wq_r_sb, wq_i_sb, bq_sb, qt_d),
                        (wk_r_sb, wk_i_sb, bk_sb, kt_d),
                    ):
                        psA = ps_qk.tile([64, TCH], F32, name="psA", tag="psA")
                        psBf = ps_qk.tile([P, TCH], F32, name="psB", tag="psB")
                        psB = psBf[64:128]
                        for d in range(DT):
                            nc.tensor.matmul(
                                psA,
                                _mm(wr[:, d, h * DH:(h + 1) * DH]),
                                _mm(xrT[:, d, :]),
                                start=(d == 0), stop=(d == DT - 1))
                            nc.tensor.matmul(
                                psB,
                                _mm(wi[:, d, h * DH:(h + 1) * DH]),
                                _mm(xiT[:, d, :]),
                                start=(d == 0), stop=(d == DT - 1),
                                tile_position=(0, 64))
                        q_sb = sqk.tile([P, TCH], MD, name="q_sb")
                        nc.any.tensor_scalar_add(
                            out=q_sb[0:64], in0=psA, scalar1=bias[0:64, h:h + 1])
                        nc.any.tensor_scalar_add(
                            out=q_sb[64:128], in0=psB,
                            scalar1=bias[64:128, h:h + 1])
                        nc.sync.dma_start(
                            dstd[h, :, tch * TCH:(tch + 1) * TCH], q_sb)

                # V token-major: psum [tok(128), CW] for r and i, then pack
                # v_sb[:, kt, head*128 + (vr|vi)]
                for s in range(TS):
                    ktile = tch * TS + s
                    pvr = ps_v.tile([P, CW], F32, name="pvr", tag="pv")
                    nc.tensor.matmul(pvr, _mm(ones_row), _mm(bvr_sb),
                                     start=True, stop=False)
                    for d in range(DT):
                        nc.tensor.matmul(
                            pvr, _mm(xrT[:, d, s * P:(s + 1) * P]),
                            _mm(wv_r_sb[:, d, :]),
                            start=False, stop=(d == DT - 1))
                    pvi = ps_v.tile([P, CW], F32, name="pvi", tag="pv")
                    nc.tensor.matmul(pvi, _mm(ones_row), _mm(bvi_sb),
                                     start=True, stop=False)
                    for d in range(DT):
                        nc.tensor.matmul(
                            pvi, _mm(xiT[:, d, s * P:(s + 1) * P]),
                            _mm(wv_i_sb[:, d, :]),
                            start=False, stop=(d == DT - 1))
                    for h in range(HPC):
                        nc.any.tensor_copy(
                            out=v_sb[:, ktile, h * P:h * P + 64],
                            in_=pvr[:, h * DH:(h + 1) * DH])
                        nc.any.tensor_copy(
                            out=v_sb[:, ktile, h * P + 64:(h + 1) * P],
                            in_=pvi[:, h * DH:(h + 1) * DH])

        # ================= Phase 2: causal attention =================
        with ExitStack() as ctx:
            qk_in = ctx.enter_context(tc.tile_pool(name="qk_in", bufs=2))
            epool = ctx.enter_context(tc.tile_pool(name="epool", bufs=6))
            rpool = ctx.enter_context(tc.tile_pool(name="rpool", bufs=2))
            ps_s = ctx.enter_context(tc.tile_pool(name="ps_s", bufs=3, space="PSUM"))
            ps_o = ctx.enter_context(tc.tile_pool(name="ps_o", bufs=1, space="PSUM"))
            ps_l = ctx.enter_context(tc.tile_pool(name="ps_l", bufs=1, space="PSUM"))
            ps_b = ctx.enter_context(tc.tile_pool(name="ps_b", bufs=1, space="PSUM"))

            for h in range(HPC):
                pair, lo = h // 2, h % 2
                base_r = 64 * lo          # vr rows in ORT[pair]
                base_i = 64 * (1 - lo)    # vi rows in OIT[pair] (swapped)
                qh = qk_in.tile([P, T], MD, name="qh")
                nc.sync.dma_start(qh, qt_d[h])
                kh = qk_in.tile([P, T], MD, name="kh")
                nc.sync.dma_start(kh, kt_d[h])

                for j in range(QC):
                    nk = (j + 1) * QKB
                    po_r = ps_o.tile([P, QCH], F32, name="po_r")
                    po_i = ps_o.tile([P, QCH], F32, name="po_i")
                    pl = ps_l.tile([1, QCH], F32, name="pl")
                    for k in range(nk):
                        st = ps_s.tile([P, QCH], F32, name="st")
                        nc.tensor.matmul(
                            st, _mm(kh[:, k * P:(k + 1) * P]),
                            _mm(qh[:, j * QCH:(j + 1) * QCH]),
                            start=True, stop=True)
                        et = epool.tile([P, QCH], MD, name="et")
                        nc.scalar.activation(
                            et, st, mybir.ActivationFunctionType.Exp,
                            scale=scale)
                        if k >= j * QKB:
                            # keep where qtok >= ktok:
                            #   -p + f + (QCH*j - 128*k) >= 0
                            nc.gpsimd.affine_select(
                                out=et, in_=et,
                                compare_op=mybir.AluOpType.is_ge,
                                fill=0.0,
                                base=QCH * j - P * k,
                                pattern=[[1, QCH]],
                                channel_multiplier=-1)
                        nc.tensor.matmul(
                            pl, _mm(ones_col), _mm(et),
                            start=(k == 0), stop=(k == nk - 1))
                        nc.tensor.matmul(
                            po_r[base_r:base_r + 64],
                            _mm(v_sb[:, k, h * P:h * P + 64]), _mm(et),
                            start=(k == 0), stop=(k == nk - 1),
                            tile_position=(0, base_r))
                        nc.tensor.matmul(
                            po_i[base_i:base_i + 64],
                            _mm(v_sb[:, k, h * P + 64:(h + 1) * P]), _mm(et),
                            start=(k == 0), stop=(k == nk - 1),
                            tile_position=(0, base_i))
                    rl = rpool.tile([1, QCH], MD, name="rl")
                    with nc.allow_low_precision(
                            reason="1/l in f32r feeds f32r bcast matmul"):
                        nc.vector.reciprocal(rl, pl)
                    pb = ps_b.tile([P, QCH], F32, name="pb")
                    nc.tensor.matmul(pb, _mm(ones_row), _mm(rl),
                                     start=True, stop=True)
                    sb_b = rpool.tile([P, QCH], F32, name="sb_b")
                    nc.any.tensor_copy(out=sb_b, in_=pb)
                    qs = slice(j * QCH, (j + 1) * QCH)
                    nc.any.tensor_mul(
                        out=ort[pair][base_r:base_r + 64, qs],
                        in0=po_r[base_r:base_r + 64],
                        in1=sb_b[base_r:base_r + 64])
                    nc.any.tensor_mul(
                        out=oit[pair][base_i:base_i + 64, qs],
                        in0=po_i[base_i:base_i + 64],
                        in1=sb_b[base_i:base_i + 64])

        # ================= Phase 3: output projection =================
        with ExitStack() as ctx:
            wop = ctx.enter_context(tc.tile_pool(name="wop", bufs=1))
            sout = ctx.enter_context(tc.tile_pool(name="sout", bufs=3))
            ps_f = ctx.enter_context(tc.tile_pool(name="ps_f", bufs=2, space="PSUM"))

            wor_sb = wop.tile([P, NPAIR, D], MD, name="wor_sb")
            nc.sync.dma_start(
                wor_sb, _dm(wo_r.rearrange("(t p) m -> p t m", p=P)))
            woi_sb = wop.tile([P, NPAIR, D], MD, name="woi_sb")
            nc.sync.dma_start(
                woi_sb, _dm(wo_i.rearrange("(t p) m -> p t m", p=P)))

            NC2 = D // 512
            for (oblocks, wsb, odst) in (
                (ort, wor_sb, out_r_t), (oit, woi_sb, out_i_t)
            ):
                for t in range(KT):
                    for n in range(NC2):
                        pf = ps_f.tile([P, 512], F32, name="pf")
                        for kk in range(NPAIR):
                            nc.tensor.matmul(
                                pf,
                                _mm(oblocks[kk][:, t * P:(t + 1) * P]),
                                _mm(wsb[:, kk, n * 512:(n + 1) * 512]),
                                start=(kk == 0), stop=(kk == NPAIR - 1))
                        ot = sout.tile([P, 512], F32, name="ot")
                        nc.any.tensor_copy(out=ot, in_=pf)
                        nc.sync.dma_start(
                            odst[:, t, n * 512:(n + 1) * 512], ot)

    nc.compile()
    return nc


def make_core_inputs(inputs, cfg=CFG):
    """Slice full inputs into 8 per-core input maps."""
    HPC, DH = cfg["HPC"], cfg["DH"]
    CW = HPC * DH
    f = lambda a: np.ascontiguousarray(np.asarray(a, dtype=np.float32))
    x_real, x_imag = f(inputs["x_real"]), f(inputs["x_imag"])
    maps = []
    for c in range(N_CORES):
        b = c // 4
        g = c % 4
        cs = slice(g * CW, (g + 1) * CW)
        bqr, bqi = f(inputs["bqr"])[cs], f(inputs["bqi"])[cs]
        bkr, bki = f(inputs["bkr"])[cs], f(inputs["bki"])[cs]
        bq_t = np.stack(
            [np.concatenate([bqr[h * DH:(h + 1) * DH], bqi[h * DH:(h + 1) * DH]])
             for h in range(HPC)], axis=1)
        bk_t = np.stack(
            [np.concatenate([bkr[h * DH:(h + 1) * DH], bki[h * DH:(h + 1) * DH]])
             for h in range(HPC)], axis=1)
        woi = f(inputs["Woi"])[cs, :]
        # OIT pair rows are [h_odd ; h_even] -> permute wo_i rows to match
        woi_perm = np.concatenate(
            [np.concatenate([woi[2 * p * DH + DH:2 * p * DH + 2 * DH],
                             woi[2 * p * DH:2 * p * DH + DH]])
             for p in range(HPC // 2)])
        maps.append({
            "x_r": x_real[b], "x_i": x_imag[b],
            "wq_r": f(inputs["Wqr"])[:, cs], "wq_i": f(inputs["Wqi"])[:, cs],
            "wk_r": f(inputs["Wkr"])[:, cs], "wk_i": f(inputs["Wki"])[:, cs],
            "wv_r": f(inputs["Wvr"])[:, cs], "wv_i": f(inputs["Wvi"])[:, cs],
            "wo_r": f(inputs["Wor"])[cs, :], "wo_i": np.ascontiguousarray(woi_perm),
            "bq": np.ascontiguousarray(bq_t), "bk": np.ascontiguousarray(bk_t),
            "bv_r": f(inputs["bvr"])[None, cs], "bv_i": f(inputs["bvi"])[None, cs],
        })
    return maps


def kernel(**inputs):
    global LAST
    nc = build_program(CFG)
    in_maps = make_core_inputs(inputs)
    res = run_bass_kernel_spmd(
        nc, in_maps, core_ids=list(range(N_CORES)), trace=TRACE)
    LAST = {"exec_time_ns": res.exec_time_ns,
            "trace": res.instructions_and_trace,
            "profile_json": res.profile_json,
            "nc": nc}
    f = lambda a: np.asarray(a, dtype=np.float32)
    bor, boi = f(inputs["bor"]), f(inputs["boi"])
    final_r = np.stack([
        sum(res.results[c]["out_r"] for c in range(4 * b, 4 * b + 4)) + bor
        for b in range(B)]).astype(np.float32)
    final_i = np.stack([
        sum(res.results[c]["out_i"] for c in range(4 * b, 4 * b + 4)) + boi
        for b in range(B)]).astype(np.float32)
    return final_r, final_i

